# revision 25
# baseline (speedup 1.0000x reference)
"""GCN graph-classification kernel for 8 Trainium2 NeuronCores (v2).

Model (PyG-style GCNConv x2 + mean pool + log_softmax):
    h   = x @ W1
    H1  = relu(Ahat @ h + b1)          Ahat = D^-1/2 (A + I) D^-1/2
    H2  = Ahat @ (H1 @ W2) + b2
    out = log_softmax(mean-pool-per-graph(H2))

v2 distribution strategy (8 cores):
  * nodes dealt snake-wise by in-degree across cores (edge balance);
    within a core ordered degree-descending (6250/core + 22 pad).
  * h = dis*(x@W1) computed locally in bf16; kept in SBUF and stored
    to DRAM once.  Two chained AllGathers publish the degree-heavy
    "lo" position range (pos<3200) first, then the "hi" range, so
    remote gathers can start as soon as the lo half has landed.
  * layer-1 aggregation via gpsimd dma_gather of per-edge source rows
    (bf16, 256B/row) + one-hot selector matmuls accumulating in PSUM.
    Three gather phases ordered own (src on this core, needs no
    AllGather -- overlaps the collective+startup barrier), lo, hi.
    Partial per-tile sums are spilled to SBUF (bf16) between phases
    and re-injected with identity matmuls.  Chunks are 128 rows and
    may straddle adjacent destination tiles (extra dloc column per
    straddle) so padding only occurs per (phase, batch) group.
    Self-loops never gather: identity matmul on the local h tile.
  * layer 2 + mean pooling folded:  pooled = (Q @ H1) @ W2 + b2  with
    Q = P_mean @ Ahat  (dense per-node-tile blocks, built on host,
    bf16).  Per-graph partial sums get W2 applied locally and only
    [16 x 512] floats are AllReduced.  log_softmax on 4 tiles.
"""

import os
import numpy as np

import concourse.bacc as bacc
import concourse.mybir as mybir
from concourse import tile
from concourse.bass_utils import run_bass_kernel_spmd

# ---------------------------------------------------------------- constants
N, E, F, HID, C, G = 50000, 600000, 128, 128, 16, 500
P = 8                      # NeuronCores
NV = N // P                # nodes per core (6250)
NT = 49                    # node tiles per core
TPAD = NT * 128            # padded per-core node count (6272)
GP = 512                   # padded graph count
GT = GP // 128             # graph tiles
NB = 7                     # batches (NT % NB == 0)
LOP = 2176                 # "lo" positions per core (17 tiles); keeps
                           # slot_hi = core*4096 + (pos-LOP) <= 32767 (int16)
HIP = TPAD - LOP           # "hi" positions per core (4096, 32 tiles)

AF = mybir.ActivationFunctionType
ALU = mybir.AluOpType

LAST_EXEC_NS = None
LAST_RESULT = None


def _install_profile_hook():
    import sys
    import types
    if "antenv.axon_hooks" in sys.modules:
        return True
    try:
        from trn_agent_boot.trn_boot import _ntff_profile_via_ctypes
        hook = _ntff_profile_via_ctypes("/opt/axon/libaxon_pjrt.so")
        if hook is None:
            return False
        mod = types.ModuleType("antenv.axon_hooks")
        mod._hook = hook
        mod.get_axon_ntff_profile_hook = lambda: mod._hook

        def _set(h):
            mod._hook = h
        mod.set_axon_ntff_profile_hook = _set
        sys.modules["antenv.axon_hooks"] = mod
        import antenv
        antenv.axon_hooks = mod
        return True
    except Exception as e:  # profiling is best-effort
        print(f"profile hook unavailable: {e}")
        return False


# ---------------------------------------------------------------- host prep
def _preprocess(x, W1, b1, W2, b2, edge_src, edge_dst, batch):
    import ml_dtypes
    f32 = np.float32
    src = np.asarray(edge_src, np.int64)
    dst = np.asarray(edge_dst, np.int64)
    bat = np.asarray(batch, np.int64)
    x = np.asarray(x, f32)

    deg = np.bincount(dst, minlength=N).astype(np.float64) + 1.0
    dis = 1.0 / np.sqrt(deg)
    cnt = np.maximum(np.bincount(bat, minlength=G), 1).astype(np.float64)
    odeg = np.bincount(src, minlength=N).astype(np.int64)

    # snake-deal nodes to cores by IN-degree (balances per-core edge
    # counts, which are sums of in-degrees over the core's nodes)
    rank = np.argsort(-deg, kind="stable")
    snake = np.empty(16, np.int64)
    snake[:8] = np.arange(8)
    snake[8:] = np.arange(7, -1, -1)
    core_of_rank = snake[np.arange(N) % 16]
    core_of = np.empty(N, np.int64)
    core_of[rank] = core_of_rank
    # within a core order by OUT-degree so the early "lo" position range
    # carries most of the gather traffic (published by the first AllGather)
    pos = np.empty(N, np.int64)
    order = np.empty((P, NV), np.int64)    # order[k, j] = node at pos j
    for k in range(P):
        nodes_k = rank[core_of_rank == k]
        nodes_k = nodes_k[np.argsort(-odeg[nodes_k], kind="stable")]
        order[k] = nodes_k
        pos[nodes_k] = np.arange(NV)

    # gather-source slot tables
    slot_own = pos                                   # [0, 6250)
    slot_lo = core_of * LOP + pos                    # valid when pos < LOP
    slot_hi = core_of * HIP + (pos - LOP)            # valid when pos >= LOP

    # ---- layer-1 gather edges (true edges only; the added self-loops are
    # an identity matmul on the local h tile inside the kernel)
    d_core = core_of[dst]
    d_pos = pos[dst]
    t_of = d_pos // 128
    seg = np.where(core_of[src] == d_core, 0, np.where(pos[src] < LOP, 1, 2))
    gidx = np.where(seg == 0, slot_own[src],
                    np.where(seg == 1, slot_lo[src], slot_hi[src]))

    # assign tiles to batches by greedy bin-packing of per-(core,seg)
    # count vectors, minimizing the padded (max-over-core) chunk total
    cnt3 = np.zeros((P, 3, NT), np.int64)
    np.add.at(cnt3, (d_core, seg, t_of), 1)
    tot_t = cnt3.sum(axis=(0, 1))
    groups = [[] for _ in range(NB)]
    gsum = np.zeros((NB, P, 3), np.int64)
    for t in np.argsort(-tot_t, kind="stable"):
        best, bcost = None, None
        for b in range(NB):
            if len(groups[b]) >= NT // NB:
                continue
            new = gsum[b] + cnt3[:, :, t]            # [P, 3]
            cost = (-(-new.max(axis=0) // 128)).sum()
            if bcost is None or cost < bcost:
                best, bcost = b, cost
        groups[best].append(int(t))
        gsum[best] += cnt3[:, :, t]
    b_of_tile = np.empty(NT, np.int64)
    rank_of_tile = np.empty(NT, np.int64)
    for b in range(NB):
        for r, t in enumerate(groups[b]):
            b_of_tile[t] = b
            rank_of_tile[t] = r
    b_of = b_of_tile[t_of]
    tile_rank_in_batch = rank_of_tile[t_of]

    # group edges by (dst core, seg, batch); inside a group order by the
    # destination tile's rank within the batch
    key = ((d_core * 3 + seg) * NB + b_of) * NT + tile_rank_in_batch
    ordr = np.argsort(key, kind="stable")
    gidx_s = gidx[ordr]
    dpos_s = d_pos[ordr]
    tof_s = t_of[ordr]
    grp_s = ((d_core * 3 + seg) * NB + b_of)[ordr]
    NG = P * 3 * NB
    bounds = np.searchsorted(grp_s, np.arange(NG + 1))
    cnts = np.diff(bounds).reshape(P, 3, NB)
    # padded chunks per (seg, batch): max over cores
    CH = (-(-cnts // 128)).max(axis=0)   # [3, NB]
    L = CH * 128                         # padded rows per group

    tiles_of_batch = groups

    # per-group metadata and per-core tables
    # table layout: groups in program order (seg-major, batch-minor)
    group_off = {}                       # (seg, b) -> idx-row offset
    off = 0
    for s in range(3):
        for b in range(NB):
            group_off[(s, b)] = off
            off += int(L[s, b])
    NIDX = off
    idx_flat = np.zeros((P, NIDX), np.int16)
    # chunk tile coverage: per group, per chunk, the set of tiles (union
    # over cores); then dloc columns per (chunk, tile)
    tiles_per_chunk = {}                 # (s, b, c) -> sorted tile list
    for s in range(3):
        for b in range(NB):
            for c in range(int(CH[s, b])):
                tiles_per_chunk[(s, b, c)] = set()
    for k in range(P):
        for s in range(3):
            for b in range(NB):
                g = (k * 3 + s) * NB + b
                g0, g1 = bounds[g], bounds[g + 1]
                n = g1 - g0
                o = group_off[(s, b)]
                idx_flat[k, o:o + n] = gidx_s[g0:g1]
                rows = np.arange(n)
                for c in np.unique(rows // 128):
                    r0, r1 = c * 128, min((c + 1) * 128, n)
                    for t in np.unique(tof_s[g0 + r0:g0 + r1]):
                        tiles_per_chunk[(s, b, int(c))].add(int(t))

    # dloc column table + per-(seg,b,tile) matmul op lists
    assert int(CH.min()) >= 1
    col_list = []                        # (s, b, c, t) in column order
    ops = {}                             # (s, b, t) -> [(c, col)]
    for s in range(3):
        for b in range(NB):
            for c in range(int(CH[s, b])):
                for t in sorted(tiles_per_chunk[(s, b, c)]):
                    col = len(col_list)
                    col_list.append((s, b, c, t))
                    ops.setdefault((s, b, t), []).append((c, col))
    # every (seg, batch, tile) needs >= 1 op so PSUM start/stop flags work;
    # point missing ones at a shared always-miss column
    dummy_col = len(col_list)
    need_dummy = False
    for s in range(3):
        for b in range(NB):
            for t in tiles_of_batch[b]:
                if (s, b, t) not in ops:
                    ops[(s, b, t)] = [(0, dummy_col)]
                    need_dummy = True
    NCOL = len(col_list) + (1 if need_dummy else 0)
    col_lookup = {}
    for col, (cs, cb, ccc, ct) in enumerate(col_list):
        col_lookup.setdefault((cs, cb), np.full((int(CH[cs, cb]), NT), -1,
                                                np.int64))[ccc, ct] = col
    dloc_all = np.full((P, 128, NCOL), -1.0, f32)
    for k in range(P):
        for s in range(3):
            for b in range(NB):
                g = (k * 3 + s) * NB + b
                g0, g1 = bounds[g], bounds[g + 1]
                n = g1 - g0
                if n == 0:
                    continue
                rows = np.arange(n)
                pp = rows % 128
                cc = rows // 128
                dl = (dpos_s[g0:g1] - tof_s[g0:g1] * 128).astype(f32)
                tt = tof_s[g0:g1]
                cols = col_lookup[(s, b)][cc, tt]
                assert cols.min() >= 0
                dloc_all[k, pp, cols] = dl
    assert idx_flat.min() >= 0
    assert int(idx_flat.max()) < 32768
    # wrap gather indices: i -> [i % 16, i // 16], replicated to 128 parts
    idxs = np.tile(
        idx_flat.reshape(P, NIDX // 16, 16).transpose(0, 2, 1), (1, 8, 1)
    ).astype(np.int16)

    # ---- per-core data arrays (dis prescale folded into x host-side)
    import ml_dtypes as mld
    xT = np.zeros((P, 128, TPAD), mld.bfloat16)
    disc = np.zeros((P, 128, NT), f32)
    qb = np.zeros((P, TPAD, GP), f32)
    xs = x * dis[:, None].astype(f32)
    for k in range(P):
        ok = order[k]
        xT[k, :, :NV] = xs[ok].T.astype(mld.bfloat16)
        d = np.zeros(TPAD, f32)
        d[:NV] = dis[ok].astype(f32)
        disc[k] = d.reshape(NT, 128).T

    # ---- layer-2 Q blocks (incl. self-loops): qb[core, pos[s], g] += v
    q_src = np.concatenate([src, np.arange(N)])
    q_dst = np.concatenate([dst, np.arange(N)])
    g_of = bat[q_dst]
    val = (dis[q_src] * dis[q_dst] / cnt[g_of]).astype(f32)
    np.add.at(qb, (core_of[q_src], pos[q_src], g_of), val)

    iota2d = np.broadcast_to(np.arange(128, dtype=f32), (128, 128)).copy()
    eye16 = np.eye(16, dtype=f32)
    eye128 = np.eye(128, dtype=ml_dtypes.bfloat16)
    qb = qb.astype(ml_dtypes.bfloat16)

    W1 = np.ascontiguousarray(np.asarray(W1, f32).astype(mld.bfloat16))
    W2 = np.ascontiguousarray(np.asarray(W2, f32))
    b1 = np.asarray(b1, f32)
    b2 = np.asarray(b2, f32)
    use_b1 = bool(np.any(b1))
    use_b2 = bool(np.any(b2))

    in_maps = []
    for k in range(P):
        m = {
            "xT": np.ascontiguousarray(xT[k]),
            "qb": np.ascontiguousarray(qb[k]),
            "idxs": np.ascontiguousarray(idxs[k]),
            "dloc": np.ascontiguousarray(dloc_all[k]),
            "disc": np.ascontiguousarray(disc[k]),
            "w1": W1, "w2": W2,
            "iota": iota2d, "eye16": eye16, "eye128": eye128,
        }
        if use_b1:
            rr = np.zeros((1, TPAD), f32)
            rr[0, :NV] = np.sqrt(deg[order[k]]).astype(f32)
            m["rdis"] = rr
            m["b1r"] = b1.reshape(1, F)
        if use_b2:
            m["b2r"] = b2.reshape(C, 1)
        in_maps.append(m)

    plan = dict(NIDX=NIDX, NCOL=NCOL, CH=CH, L=L,
                tiles_of_batch=tiles_of_batch, group_off=group_off,
                ops=ops, use_b1=use_b1, use_b2=use_b2)
    return plan, in_maps


# ---------------------------------------------------------------- bass build
def _build(plan):
    dt = mybir.dt
    f32, bf16, i16 = dt.float32, dt.bfloat16, dt.int16
    NIDX, NCOL = plan["NIDX"], plan["NCOL"]
    CH, L = plan["CH"], plan["L"]
    group_off, ops = plan["group_off"], plan["ops"]
    tiles_of_batch = plan["tiles_of_batch"]
    use_b1, use_b2 = plan["use_b1"], plan["use_b2"]

    nc = bacc.Bacc("TRN2", target_bir_lowering=False, debug=False,
                   num_devices=P)
    xT_d = nc.dram_tensor("xT", [128, TPAD], bf16, kind="ExternalInput")
    qb_d = nc.dram_tensor("qb", [TPAD, GP], bf16, kind="ExternalInput")
    idxs_d = nc.dram_tensor("idxs", [128, NIDX // 16], i16,
                            kind="ExternalInput")
    dloc_d = nc.dram_tensor("dloc", [128, NCOL], f32, kind="ExternalInput")
    disc_d = nc.dram_tensor("disc", [128, NT], f32, kind="ExternalInput")
    w1_d = nc.dram_tensor("w1", [F, HID], bf16, kind="ExternalInput")
    w2_d = nc.dram_tensor("w2", [HID, C], f32, kind="ExternalInput")
    iota_d = nc.dram_tensor("iota", [128, 128], f32, kind="ExternalInput")
    eye_d = nc.dram_tensor("eye16", [16, 16], f32, kind="ExternalInput")
    eye128_d = nc.dram_tensor("eye128", [128, 128], bf16,
                              kind="ExternalInput")
    if use_b1:
        rdis_d = nc.dram_tensor("rdis", [1, TPAD], f32, kind="ExternalInput")
        b1_d = nc.dram_tensor("b1r", [1, F], f32, kind="ExternalInput")
    if use_b2:
        b2_d = nc.dram_tensor("b2r", [C, 1], f32, kind="ExternalInput")
    y_d = nc.dram_tensor("y", [G, C], f32, kind="ExternalOutput")

    with tile.TileContext(nc) as tc:
        cpool = tc.alloc_tile_pool(name="const", bufs=1)
        dram = tc.alloc_tile_pool(name="dram", bufs=1, space="DRAM")

        # phase-B-critical loads first (xT feeds the first matmul), then
        # the gather tables (overlap phase-B compute), then the rest
        xT_sb = cpool.tile([128, TPAD], bf16)
        nc.sync.dma_start(xT_sb[:, 0:TPAD // 2], xT_d[:, 0:TPAD // 2])
        nc.sync.dma_start(xT_sb[:, TPAD // 2:], xT_d[:, TPAD // 2:])
        w1_sb = cpool.tile([F, HID], bf16)
        nc.sync.dma_start(w1_sb[:], w1_d[:, :])
        disc_sb = cpool.tile([128, NT], f32)
        nc.sync.dma_start(disc_sb[:], disc_d[:, :])
        idxs_sb = cpool.tile([128, NIDX // 16], i16)
        nc.sync.dma_start(idxs_sb[:], idxs_d[:, :])
        dloc_sb = cpool.tile([128, NCOL], f32)
        nc.sync.dma_start(dloc_sb[:], dloc_d[:, :])
        iota_sb = cpool.tile([128, 128], f32)
        nc.sync.dma_start(iota_sb[:], iota_d[:, :])
        eye128_sb = cpool.tile([128, 128], bf16)
        nc.sync.dma_start(eye128_sb[:], eye128_d[:, :])
        eye_sb = cpool.tile([16, 16], f32)
        nc.sync.dma_start(eye_sb[:], eye_d[:, :])
        w2_sb = cpool.tile([HID, C], f32)
        nc.sync.dma_start(w2_sb[:], w2_d[:, :])
        h1_sb = cpool.tile([128, TPAD], bf16)
        h_sb = cpool.tile([128, TPAD], bf16)      # prescaled h (local)
        part_sb = cpool.tile([128, TPAD], bf16)   # inter-phase partials
        if use_b1:
            rdis_sb = cpool.tile([1, TPAD], f32)
            nc.sync.dma_start(rdis_sb[:], rdis_d[:, :])
            b1_sb = cpool.tile([1, F], f32)
            nc.sync.dma_start(b1_sb[:], b1_d[:, :])
        if use_b2:
            b2_sb = cpool.tile([16, 1], f32)
            nc.sync.dma_start(b2_sb[:], b2_d[:, :])

        h_own = dram.tile([TPAD, F], bf16)
        h_lo = dram.tile([P * LOP, F], bf16)
        h_hi = dram.tile([P * HIP, F], bf16)
        ar_in = dram.tile([16, GP], f32)
        ar_out = dram.tile([16, GP], f32)

        # ---------------- phase B: h = (dis*x) @ W1 (bf16), publish
        # dis is folded into xT host-side; PSUM->SBUF copies alternate
        # between the scalar and vector engines to halve the serial chain
        with tc.tile_pool(name="hp", bufs=4, space="PSUM") as hp:
            for t in range(NT):
                ps = hp.tile([128, 128], f32)
                nc.tensor.matmul(ps[:], lhsT=xT_sb[:, t * 128:(t + 1) * 128],
                                 rhs=w1_sb[:], start=True, stop=True)
                if t % 2 == 0:
                    nc.scalar.activation(h_sb[:, t * 128:(t + 1) * 128],
                                         ps[:], AF.Copy)
                else:
                    nc.vector.tensor_copy(h_sb[:, t * 128:(t + 1) * 128],
                                          ps[:])
            nc.sync.dma_start(
                h_own[0:24 * 128, :].rearrange("(t p) f -> p t f", p=128),
                h_sb[:, 0:24 * 128].rearrange("p (t f) -> p t f", f=128))
            nc.sync.dma_start(
                h_own[24 * 128:, :].rearrange("(t p) f -> p t f", p=128),
                h_sb[:, 24 * 128:].rearrange("p (t f) -> p t f", f=128))

        # ---------------- phase C: gather phases own -> lo -> hi
        def sel_col(selp, col):
            sel = selp.tile([128, 128], bf16, tag="sel")
            nc.vector.tensor_tensor(
                out=sel[:], in0=iota_sb[:],
                in1=dloc_sb[:, col:col + 1].to_broadcast([128, 128]),
                op=ALU.is_equal)
            return sel

        # gather via an f32 view of the bf16 rows (same 256B/descriptor;
        # the Q7 desc-gen ucode runs ~20% faster with a 32-bit dtype field)
        gdt = bool(int(os.environ.get("GCN_GDT", "1")))
        state = dict(i_l2=0)

        def run_phase(s, src_ap, batches, poolT, pools, split_last=False):
            if gdt:
                src_ap = src_ap.bitcast(f32)
            gp_, selp, qp, aggp = pools
            if True:
                for bi, b in enumerate(batches):
                    nch = int(CH[s, b])
                    if nch == 0:
                        continue
                    o = group_off[(s, b)]
                    # split the final gather so its tiles' compute overlaps
                    # the second half of the gather
                    nsplit = ((nch * 3) // 4
                              if split_last and bi == len(batches) - 1 else 0)
                    pieces = []            # (c0, nch_piece, tile_buf)
                    for c0, n_ in (((0, nsplit), (nsplit, nch - nsplit))
                                   if nsplit else ((0, nch),)):
                        if n_ == 0:
                            continue
                        if gdt:
                            gt_ = gp_.tile([128, n_, 64], f32,
                                           tag=f"g{c0 > 0}")
                            esz = 64
                        else:
                            gt_ = gp_.tile([128, n_, 128], bf16,
                                           tag=f"g{c0 > 0}")
                            esz = F
                        oo = o + 128 * c0
                        nc.gpsimd.dma_gather(
                            out_ap=gt_[:], in_ap=src_ap,
                            idxs_ap=idxs_sb[:, oo // 16:
                                            (oo + 128 * n_) // 16],
                            num_idxs=n_ * 128, num_idxs_reg=n_ * 128,
                            elem_size=esz,
                            single_packet=bool(int(os.environ.get('GCN_SP', '0'))))
                        pieces.append((c0, n_, gt_))

                    def chunk_rhs(c):
                        for c0, n_, gt_ in pieces:
                            if c0 <= c < c0 + n_:
                                rhs = gt_[:, c - c0, :]
                                return rhs.bitcast(bf16) if gdt else rhs
                        raise AssertionError(c)

                    qts = []
                    for t in tiles_of_batch[b]:
                        t_ops = ops[(s, b, t)]
                        ps = aggp.tile([128, 128], f32, tag="agg")
                        if s == 0:
                            # self-loop + (optional) b1 seed
                            first = True
                            if use_b1:
                                nc.tensor.matmul(
                                    ps[:],
                                    lhsT=rdis_sb[0:1, t * 128:(t + 1) * 128],
                                    rhs=b1_sb[:], start=True, stop=False)
                                first = False
                            nc.tensor.matmul(
                                ps[:], lhsT=eye128_sb[:],
                                rhs=h_sb[:, t * 128:(t + 1) * 128],
                                start=first, stop=False)
                        else:
                            # re-inject the partial from previous phase
                            nc.tensor.matmul(
                                ps[:], lhsT=eye128_sb[:],
                                rhs=part_sb[:, t * 128:(t + 1) * 128],
                                start=True, stop=False)
                        for ci, (c, col) in enumerate(t_ops):
                            sel = sel_col(selp, col)
                            nc.tensor.matmul(
                                ps[:], lhsT=sel[:], rhs=chunk_rhs(c),
                                start=False, stop=(ci == len(t_ops) - 1))
                        if s < 2:
                            nc.scalar.activation(
                                part_sb[:, t * 128:(t + 1) * 128], ps[:],
                                AF.Copy)
                        else:
                            nc.scalar.activation(
                                h1_sb[:, t * 128:(t + 1) * 128], ps[:],
                                AF.Relu, scale=disc_sb[:, t:t + 1])
                            qt = qp.tile([128, GP], bf16, tag="q")
                            nc.sync.dma_start(
                                qt[:], qb_d[t * 128:(t + 1) * 128, :])
                            qts.append((t, qt))
                    # batch's pool contractions issued after all its tiles'
                    # selector groups, so a Q matmul waiting on a Relu never
                    # stalls the next tile's aggregation on the PE
                    if s == 2:
                        for t, qt in qts:
                            i_l2 = state["i_l2"]
                            nc.tensor.matmul(
                                poolT[:],
                                lhsT=h1_sb[:, t * 128:(t + 1) * 128],
                                rhs=qt[:],
                                start=(i_l2 == 0), stop=(i_l2 == NT - 1))
                            state["i_l2"] = i_l2 + 1
                        qts = []

        with tc.tile_pool(name="ptp", bufs=1, space="PSUM") as ptp:
            poolT = ptp.tile([128, GP], f32)
            with (
                tc.tile_pool(name="gpool", bufs=3) as gp_,
                tc.tile_pool(name="selpool", bufs=128) as selp,
                tc.tile_pool(name="qpool", bufs=6) as qp,
                tc.tile_pool(name="aggpool", bufs=7, space="PSUM") as aggp,
            ):
                pools = (gp_, selp, qp, aggp)
                # own batches 0-1 run while the cc-init barrier settles; the
                # AllGather triggers (gpsimd-only instructions) interleave
                # between own batches so lo data lands as early as possible
                run_phase(0, h_own[:, :], [0, 1, 2], poolT, pools)
                nc.gpsimd.collective_compute(
                    "AllGather", ALU.bypass, replica_groups=[list(range(P))],
                    ins=[h_own[0:LOP, :].opt()], outs=[h_lo[:].opt()])
                run_phase(0, h_own[:, :], [3], poolT, pools)
                nc.gpsimd.collective_compute(
                    "AllGather", ALU.bypass, replica_groups=[list(range(P))],
                    ins=[h_own[LOP:TPAD, :].opt()], outs=[h_hi[:].opt()])
                run_phase(0, h_own[:, :], [4, 5, 6], poolT, pools)
                run_phase(1, h_lo[:, :], list(range(NB)), poolT, pools)
                # end the hi phase (and the pipeline) on the smallest batch
                hi_order = sorted(range(NB), key=lambda b: -int(CH[2, b]))
                run_phase(2, h_hi[:, :], hi_order, poolT, pools,
                          split_last=True)

            # W2 applied locally BEFORE AllReduce: 8x less collective data
            pt_sb = cpool.tile([128, GP], f32)
            nc.scalar.activation(pt_sb[:], poolT[:], AF.Copy)
            with tc.tile_pool(name="o2p", bufs=1, space="PSUM") as o2p:
                out2 = o2p.tile([16, GP], f32)
                nc.tensor.matmul(out2[:], lhsT=w2_sb[:], rhs=pt_sb[:],
                                 start=True, stop=True)
                o2_sb = cpool.tile([16, GP], f32)
                nc.scalar.activation(o2_sb[:], out2[:], AF.Copy)
                nc.sync.dma_start(ar_in[:], o2_sb[:])

        nc.gpsimd.collective_compute(
            "AllReduce", ALU.add, replica_groups=[list(range(P))],
            ins=[ar_in[:].opt()], outs=[ar_out[:].opt()])

        # ---------------- phase D: bias, log_softmax
        with (
            tc.tile_pool(name="fin", bufs=1) as fin,
            tc.tile_pool(name="fps", bufs=2, space="PSUM") as fps,
            tc.tile_pool(name="sm", bufs=4) as smp,
        ):
            logitsT = fin.tile([16, GP], f32)
            nc.sync.dma_start(logitsT[:], ar_out[:])
            if use_b2:
                nc.vector.tensor_scalar(logitsT[:], logitsT[:],
                                        b2_sb[:, 0:1], None, ALU.add)
            # grouped by op so the scalar engine loads each activation
            # table (Exp, Ln) once instead of once per tile
            ngt = min(GT, -(-G // 128))
            tps, nmxs, exs, sms, lses = [], [], [], [], []
            for gt in range(ngt):
                tp = fps.tile([128, 16], f32, tag=f"tp{gt}")
                nc.tensor.transpose(
                    tp[:], logitsT[:, gt * 128:(gt + 1) * 128], eye_sb[:])
                tps.append(tp)
                nmx = smp.tile([128, 1], f32, tag=f"nmx{gt}")
                nc.vector.reduce_max(out=nmx[:], in_=tp[:],
                                     axis=mybir.AxisListType.X, negate=True)
                nmxs.append(nmx)
            for gt in range(ngt):
                ex = smp.tile([128, 16], f32, tag=f"ex{gt}")
                nc.scalar.activation(ex[:], tps[gt][:], AF.Exp,
                                     bias=nmxs[gt][:, 0:1])
                exs.append(ex)
            for gt in range(ngt):
                sm = smp.tile([128, 1], f32, tag=f"sm{gt}")
                nc.vector.reduce_sum(out=sm[:], in_=exs[gt][:],
                                     axis=mybir.AxisListType.X)
                sms.append(sm)
            for gt in range(ngt):
                lse = smp.tile([128, 1], f32, tag=f"lse{gt}")
                nc.scalar.activation(lse[:], sms[gt][:], AF.Ln)
                lses.append(lse)
            for gt in range(ngt):
                res = smp.tile([128, 16], f32, tag=f"res{gt}")
                nc.vector.tensor_scalar(res[:], tps[gt][:], nmxs[gt][:, 0:1],
                                        lses[gt][:, 0:1],
                                        ALU.add, ALU.subtract)
                rows = min(128, G - gt * 128)
                nc.sync.dma_start(y_d[gt * 128:gt * 128 + rows, :],
                                  res[0:rows, :])
        dram.release()
        cpool.release()
    nc.compile()
    return nc


# ---------------------------------------------------------------- entry
def kernel(x, W1, b1, W2, b2, edge_src, edge_dst, batch):
    global LAST_EXEC_NS, LAST_RESULT
    plan, in_maps = _preprocess(x, W1, b1, W2, b2,
                                edge_src, edge_dst, batch)
    nc = _build(plan)
    trace = bool(int(os.environ.get("GCN_TRACE", "0")))
    kw = {}
    if trace and _install_profile_hook():
        kw = dict(trace=True, trace_cores=[0])
    res = run_bass_kernel_spmd(nc, in_maps, core_ids=list(range(P)), **kw)
    LAST_RESULT = res
    LAST_EXEC_NS = res.exec_time_ns
    return np.ascontiguousarray(res.results[0]["y"].astype(np.float32))
